# revision 16
# baseline (speedup 1.0000x reference)
"""Trainium2 Bass kernel for BoxMultiHeadedAttention (B=4, S=1024, D=1024, H=16).

Reference math (eval mode, mask is all-ones so the masking is a no-op):
    qg/kg/qa/ka/va = per-head projections of the five inputs
    q = concat([qa, qg], -1); k = concat([ka, kg], -1)           # [B,H,S,128]
    p = softmax(q @ k.T / sqrt(128)); x = (p @ va) -> [B,S,D]
    out = sigmoid(concat([query_a, query_g], -1) @ Wgate.T + bgate) * (x @ Winfo.T + binfo)

Sharding: 8 cores = 4 batches x 2 head-halves. Core c handles batch c//2 and
heads (c%2)*8 .. +8 (which are also x-columns (c%2)*512..+512).  The GLU is
column-sharded the same way; attention-output halves are exchanged between
core pairs with per-2-head-block pairwise AllGathers.

v3 schedule:
  * scores in bf16; exp batched over [128,1024] PSUM reads.
  * software pipeline: q/k projections for head h+1 are interleaved into
    attention head h, so the ACT exp chain starts as soon as the first two
    heads are projected instead of after all eight.
  * the recip/normalize chain of head h is deferred into head h+1 (after its
    first exp) so ACT stays dense across head boundaries.
  * prioritized DMA: q/k inputs first in coarse chunks, va/wi/gate weights
    later; all biases packed into one small tensor.
  * info contraction split local/remote: my own x blocks are consumed from
    SBUF (no collective round trip) against host-permuted Winfo tiles; both
    gather rows are contracted (the self-echo row gets zero weights).
"""

import os

import ml_dtypes
import numpy as np

import concourse.bass as bass
import concourse.mybir as mybir
import concourse.tile as tile
from concourse import bacc, bass_utils

B, S, D, H = 4, 1024, 1024, 16
DK = D // H            # 64
CD = 2 * DK            # 128 concat head dim
HL = H // 2            # 8 local heads per core
T = D // 128           # 8 partition tiles per 1024 dim
NQ = S // 512          # 2 moving-dim blocks
SCALE = 1.0 / float(np.sqrt(2 * DK))

F32 = mybir.dt.float32
BF16 = mybir.dt.bfloat16
NPBF16 = ml_dtypes.bfloat16

REPLICA_GROUPS = [[0, 1], [2, 3], [4, 5], [6, 7]]


def build_nc():
    nc = bacc.Bacc("TRN2", target_bir_lowering=False, debug=False, num_devices=8)

    # ---- DRAM I/O (per-core tensors; same program on all 8 cores) ----
    d_xqa = nc.dram_tensor("xqa", [128, T * S], BF16, kind="ExternalInput")
    d_xqg = nc.dram_tensor("xqg", [128, T * S], BF16, kind="ExternalInput")
    d_xka = nc.dram_tensor("xka", [128, T * S], BF16, kind="ExternalInput")
    d_xkg = nc.dram_tensor("xkg", [128, T * S], BF16, kind="ExternalInput")
    d_xv = nc.dram_tensor("xv", [128, T * S], BF16, kind="ExternalInput")
    d_wqa = nc.dram_tensor("wqa", [128, T * 512], BF16, kind="ExternalInput")
    d_wqg = nc.dram_tensor("wqg", [128, T * 512], BF16, kind="ExternalInput")
    d_wka = nc.dram_tensor("wka", [128, T * 512], BF16, kind="ExternalInput")
    d_wkg = nc.dram_tensor("wkg", [128, T * 512], BF16, kind="ExternalInput")
    d_wv = nc.dram_tensor("wv", [128, T * 512], BF16, kind="ExternalInput")
    d_wg = nc.dram_tensor("wg", [128, 2 * T * 512], BF16, kind="ExternalInput")
    # 12 contraction tiles: 0-3 my x blocks, 4-11 the two gather rows per block
    d_wi = nc.dram_tensor("wi", [128, 12 * 512], BF16, kind="ExternalInput")
    # packed biases: cols 0:8 bq, 8:16 bk, 16:20 bgate, 20:24 binfo_eff
    d_ball = nc.dram_tensor("ball", [128, 24], F32, kind="ExternalInput")
    d_out = nc.dram_tensor("out", [4, 128, S], F32, kind="ExternalOutput")

    with tile.TileContext(nc) as tc:
        with (
            tc.tile_pool(name="xin", bufs=1) as p_xin,
            tc.tile_pool(name="wts", bufs=1) as p_w,
            tc.tile_pool(name="big", bufs=1) as p_big,
            tc.tile_pool(name="att", bufs=1) as p_att,
            tc.tile_pool(name="tail", bufs=1) as p_tail,
            tc.tile_pool(name="psA", bufs=1, space="PSUM") as p_psA,
            tc.tile_pool(name="psS", bufs=1, space="PSUM") as p_psS,
            tc.tile_pool(name="psX", bufs=1, space="PSUM") as p_psX,
            tc.tile_pool(name="dram", bufs=1, space="DRAM") as p_dram,
        ):
            # --- persistent sbuf tiles ---
            t_xqa = p_xin.tile([128, T, S], BF16, tag="qin", bufs=2)
            t_xqg = p_xin.tile([128, T, S], BF16, tag="qin", bufs=2)
            t_xka = p_xin.tile([128, T, S], BF16, tag="kin", bufs=2)
            t_xkg = p_xin.tile([128, T, S], BF16, tag="kin", bufs=2)
            t_xv = p_xin.tile([128, T, S], BF16, tag="vin", bufs=1)

            t_wqa = p_w.tile([128, T, 512], BF16, tag="w8", bufs=4)
            t_wqg = p_w.tile([128, T, 512], BF16, tag="w8", bufs=4)
            t_wka = p_w.tile([128, T, 512], BF16, tag="w8", bufs=4)
            t_wkg = p_w.tile([128, T, 512], BF16, tag="w8", bufs=4)
            t_wv = p_w.tile([128, T, 512], BF16, tag="w8", bufs=4)
            t_wi = p_w.tile([128, 12, 512], BF16, tag="wi", bufs=1)
            t_ball = p_w.tile([128, 24], F32, tag="bias", bufs=1)

            t_qT = p_big.tile([128, HL, S], BF16, tag="qk", bufs=2)
            t_kT = p_big.tile([128, HL, S], BF16, tag="qk", bufs=2)

            t_va = p_att.tile([128, T, HL, DK + 1], BF16, tag="va", bufs=1)
            # my normalized x blocks [xdim, block, seq]; lives in a weight
            # slot that frees up once the upfront q-projections finish
            t_xt = p_w.tile([128, 4, S], BF16, tag="w8", bufs=4)
            # gathered remote rows: [xdim, block, row(2), seq]
            t_xr = p_xin.tile([128, 4, 2, S], BF16, tag="kin", bufs=2)
            t_gate = p_big.tile([128, 4, S], BF16, tag="gate", bufs=1)
            # info partial sums (local half, bias folded in)
            t_part = p_big.tile([128, 4, NQ, 512], BF16, tag="part", bufs=1)

            # --- prioritized loads: q/k inputs first, the rest later ---
            def load(dt_, tl, n_t, chunk):
                r = dt_.ap().rearrange("p (t n) -> p t n", t=n_t)
                for tt in range(0, n_t, chunk):
                    nc.sync.dma_start(tl[:, tt:tt + chunk, :],
                                      r[:, tt:tt + chunk, :])

            nc.sync.dma_start(t_ball[:], d_ball.ap())
            load(d_wqa, t_wqa, T, 4)
            load(d_wqg, t_wqg, T, 4)
            load(d_xqa, t_xqa, T, 4)
            load(d_xqg, t_xqg, T, 4)
            load(d_wka, t_wka, T, 4)
            load(d_wkg, t_wkg, T, 4)
            load(d_xka, t_xka, T, 4)
            load(d_xkg, t_xkg, T, 4)
            load(d_wv, t_wv, T, 8)
            load(d_xv, t_xv, T, 4)
            load(d_wi, t_wi, 12, 12)

            # --- PE warmup: keep HAM un-throttled during the DMA lead-in ---
            t_wu = p_att.tile([128, 512], BF16, tag="wu", bufs=1)
            nc.vector.memset(t_wu[:], 0.0)
            for _ in range(12):
                pwu = p_psA.tile([128, 512], F32, tag="proj", bufs=2)
                nc.tensor.matmul(pwu[:], t_wu[:, 0:128], t_wu[:],
                                 start=True, stop=True)

            nc.vector.memset(t_va[:, :, :, DK:DK + 1], 1.0)

            # --- q/k projection groups (concurrent M=64 column-group pairs).
            # gidx: 0=(q,n0) 1=(q,n1) 2=(k,n0) 3=(k,n1) ---
            def proj_group(h, gidx):
                if gidx < 2:
                    wa, wb, xa, xb, dst, boff = (
                        t_wqa, t_wqg, t_xqa, t_xqg, t_qT, 0)
                else:
                    wa, wb, xa, xb, dst, boff = (
                        t_wka, t_wkg, t_xka, t_xkg, t_kT, 8)
                n = gidx % 2
                ps = p_psA.tile([128, 512], F32, tag="proj", bufs=2)
                for kt in range(T):
                    nc.tensor.matmul(
                        ps[0:64, :],
                        wa[:, kt, h * DK:(h + 1) * DK],
                        xa[:, kt, n * 512:(n + 1) * 512],
                        start=(kt == 0), stop=(kt == T - 1),
                        tile_position=(0, 0), skip_group_check=True,
                    )
                    nc.tensor.matmul(
                        ps[64:128, :],
                        wb[:, kt, h * DK:(h + 1) * DK],
                        xb[:, kt, n * 512:(n + 1) * 512],
                        start=(kt == 0), stop=(kt == T - 1),
                        tile_position=(0, 64), skip_group_check=True,
                    )
                nc.vector.tensor_scalar_add(
                    dst[:, h, n * 512:(n + 1) * 512], ps[:],
                    t_ball[:, boff + h:boff + h + 1],
                )

            def va_unit(st):
                ps = p_psA.tile([128, 512], F32, tag="proj", bufs=2)
                for kt in range(T):
                    nc.tensor.matmul(
                        ps[:],
                        t_xv[:, kt, st * 128:(st + 1) * 128],
                        t_wv[:, kt, :],
                        start=(kt == 0), stop=(kt == T - 1),
                    )
                nc.vector.tensor_copy(
                    t_va[:, st, :, 0:DK],
                    ps[:].rearrange("p (h d) -> p h d", h=HL),
                )

            # gate weights reuse the xv slot once va is done; loaded lazily
            t_wg = p_xin.tile([128, 2 * T, 512], BF16, tag="vin", bufs=1)
            _wg_loaded = [False]

            def gate_unit(mt, n):
                if not _wg_loaded[0]:
                    _wg_loaded[0] = True
                    load(d_wg, t_wg, 2 * T, 16)
                ps = p_psA.tile([128, 512], F32, tag="proj", bufs=2)
                for kt in range(2 * T):
                    xsrc = t_xqa if kt < T else t_xqg
                    nc.tensor.matmul(
                        ps[:],
                        t_wg[:, kt, mt * 128:(mt + 1) * 128],
                        xsrc[:, kt % T, n * 512:(n + 1) * 512],
                        start=(kt == 0), stop=(kt == 2 * T - 1),
                    )
                nc.vector.tensor_scalar_add(
                    t_gate[:, mt, n * 512:(n + 1) * 512], ps[:],
                    t_ball[:, 16 + mt:17 + mt],
                )

            gate_q = [(mt, n) for mt in range(4) for n in range(NQ)]
            _gate_pos = [0]

            def emit_gate(k=1):
                while k > 0 and _gate_pos[0] < len(gate_q):
                    mt, n = gate_q[_gate_pos[0]]
                    _gate_pos[0] += 1
                    gate_unit(mt, n)
                    k -= 1

            # local-info partials: one matmul + one DVE accumulate per
            # (block, mt, n).  Bias folded in at block 0.
            def local_info_block(b):
                for mt in range(4):
                    for n in range(NQ):
                        ps = p_psA.tile([128, 512], F32, tag="proj", bufs=2)
                        nc.tensor.matmul(
                            ps[:],
                            t_wi[:, b, mt * 128:(mt + 1) * 128],
                            t_xt[:, b, n * 512:(n + 1) * 512],
                            start=True, stop=True,
                        )
                        if b == 0:
                            nc.vector.tensor_scalar_add(
                                t_part[:, mt, n, :], ps[:],
                                t_ball[:, 20 + mt:21 + mt],
                            )
                        else:
                            nc.vector.tensor_tensor(
                                t_part[:, mt, n, :], ps[:],
                                t_part[:, mt, n, :], op=mybir.AluOpType.add,
                            )

            def ship_block(i):
                cc_in = p_dram.tile([1, 128, S], BF16, name=f"cci_{i}")
                cc_out = p_dram.tile([2, 128, S], BF16, name=f"cco_{i}")
                nc.sync.dma_start(cc_in[0], t_xt[:, i, :])
                nc.gpsimd.collective_compute(
                    "AllGather", mybir.AluOpType.bypass,
                    replica_groups=REPLICA_GROUPS,
                    ins=[cc_in[:].opt()], outs=[cc_out[:].opt()],
                )
                nc.sync.dma_start(t_xr[:, i, 0, :], cc_out[0])
                nc.sync.dma_start(t_xr[:, i, 1, :], cc_out[1])

            # finisher for head h: recip chain + normalize (+ ship after odd
            # heads).  Deferred into the next head so ACT stays dense.
            def make_finisher(h, t_px):
                def fin():
                    t_ln = p_att.tile([1, S], F32, tag="recip", bufs=1,
                                      name=f"ln_{h}")
                    nc.scalar.activation(t_ln[:], t_px[DK:DK + 1, :],
                                         mybir.ActivationFunctionType.Ln)
                    t_recip = p_att.tile([1, S], BF16, tag="recip2", bufs=1,
                                         name=f"recip_{h}")
                    nc.scalar.activation(t_recip[:], t_ln[:],
                                         mybir.ActivationFunctionType.Exp,
                                         scale=-1.0)
                    t_bc = p_att.tile([DK, S], BF16, tag="bc", bufs=1,
                                      name=f"bc_{h}")
                    nc.gpsimd.partition_broadcast(t_bc[:], t_recip[:])
                    nc.vector.tensor_tensor(
                        t_xt[(h % 2) * DK:(h % 2) * DK + DK, h // 2, :],
                        t_px[0:DK, :], t_bc[:], op=mybir.AluOpType.mult,
                    )
                    if h % 2 == 1:
                        ship_block(h // 2)
                        local_info_block(h // 2)
                return fin

            pend = []

            def attn_head(h):
                px = p_psX.tile([DK + 1, S], F32, tag="x", bufs=1,
                                name=f"px_{h}")
                tes = []
                pv_done = [0]

                def pv(j):
                    for n in range(NQ):
                        nc.tensor.matmul(
                            px[:, n * 512:(n + 1) * 512],
                            t_va[:, j, h, :],
                            tes[j][:, n * 512:(n + 1) * 512],
                            start=(j == 0), stop=(j == T - 1),
                        )
                    pv_done[0] = j + 1

                for kt in range(T):
                    pss = p_psS.tile([128, S], F32, tag="s", bufs=2,
                                     name=f"pss_{h}_{kt}")
                    for n in range(NQ):
                        nc.tensor.matmul(
                            pss[:, n * 512:(n + 1) * 512],
                            t_kT[:, h, kt * 128:(kt + 1) * 128],
                            t_qT[:, h, n * 512:(n + 1) * 512],
                            start=True, stop=True,
                        )
                    te = p_att.tile([128, S], BF16, tag="exp", bufs=3,
                                    name=f"te_{h}_{kt}")
                    nc.scalar.activation(
                        te[:], pss[:],
                        mybir.ActivationFunctionType.Exp, scale=SCALE,
                    )
                    tes.append(te)
                    if kt == 1 and len(pend) >= 2:
                        pend.pop(0)()
                    if h == 0:
                        # va tiles are produced just in time inside head 0;
                        # p@v for key-tile j must follow va tile j in the PE
                        # queue.
                        if kt == 2:
                            va_unit(0); va_unit(1); va_unit(2)
                            pv(0); pv(1)
                        elif kt >= 3:
                            va_unit(kt)
                            pv(kt - 1)
                    else:
                        if kt > 0:
                            pv(kt - 1)
                        # pipeline: k-projections of heads 2-7 run inside
                        # heads 1-3; gate preacts inside heads 4-7
                        if 1 <= h <= 3:
                            nh = 2 * h
                            if kt == 1:
                                proj_group(nh, 2)
                            elif kt == 3:
                                proj_group(nh, 3)
                            elif kt == 5:
                                proj_group(nh + 1, 2)
                            elif kt == 6:
                                proj_group(nh + 1, 3)
                        if h >= 4 and kt in (2, 5):
                            emit_gate(1)
                while pv_done[0] < T:
                    pv(pv_done[0])
                # copy px out of PSUM immediately so its bank pair recycles
                t_px = p_att.tile([DK + 1, S], BF16, tag="pxs", bufs=3,
                                  name=f"pxs_{h}")
                nc.vector.tensor_copy(t_px[:], px[:])
                pend.append(make_finisher(h, t_px))

            # --- pipeline: all q-projections + k h0,h1 upfront (overlapping
            # the DMA lead-in), then attention with inline k-projections ---
            for h in range(HL):
                proj_group(h, 0)
                proj_group(h, 1)
            for h in (0, 1):
                proj_group(h, 2)
                proj_group(h, 3)
            for h in range(HL):
                attn_head(h)
            while pend:
                pend.pop(0)()   # finishers for heads 6,7 (ship blocks 2,3)

            # --- tail: batched gate sigmoids, remote info, GLU, store ---
            emit_gate(8)
            for mt in range(4):
                nc.scalar.activation(
                    t_gate[:, mt, :], t_gate[:, mt, :],
                    mybir.ActivationFunctionType.Sigmoid,
                )

            for mt in range(4):
                for n in range(NQ):
                    ps = p_psA.tile([128, 512], F32, tag="proj", bufs=2)
                    k = 0
                    for j in range(4):
                        for r in range(2):
                            nc.tensor.matmul(
                                ps[:],
                                t_wi[:, 4 + 2 * j + r, mt * 128:(mt + 1) * 128],
                                t_xr[:, j, r, n * 512:(n + 1) * 512],
                                start=(k == 0), stop=(k == 7),
                            )
                            k += 1
                    t_ob = p_tail.tile([128, 512], F32, tag="outb", bufs=2)
                    nc.vector.tensor_tensor(
                        t_ob[:], ps[:], t_part[:, mt, n, :],
                        op=mybir.AluOpType.add,
                    )
                    nc.vector.tensor_tensor(
                        t_ob[:], t_ob[:],
                        t_gate[:, mt, n * 512:(n + 1) * 512],
                        op=mybir.AluOpType.mult,
                    )
                    nc.sync.dma_start(
                        d_out.ap()[mt, :, n * 512:(n + 1) * 512], t_ob[:])

    nc.compile()
    return nc


def make_in_maps(inputs):
    """Host-side sharding: transpose/slice/cast the full inputs per core."""
    f32 = np.float32
    g = {k: np.asarray(v) for k, v in inputs.items()}
    binfo_eff = (
        g["binfo"].astype(np.float64)
        + g["Winfo"].astype(np.float64) @ g["bva"].astype(np.float64)
    ).astype(f32)

    in_maps = []
    for c in range(8):
        b, hh = c // 2, c % 2
        hs = slice(hh * 512, (hh + 1) * 512)

        def pmajor(a):
            rows, n = a.shape
            t = rows // 128
            return np.ascontiguousarray(
                a.reshape(t, 128, n).transpose(1, 0, 2).reshape(128, t * n))

        def xt(name):
            return pmajor(g[name][b].T.astype(NPBF16))

        def wt(name):
            return pmajor(g[name][hs].T.astype(NPBF16))

        def bqk(pa, pg):
            a = g[pa][hs].reshape(HL, DK).T.astype(f32)   # [64, 8]
            gg = g[pg][hs].reshape(HL, DK).T.astype(f32)
            return np.vstack([a, gg])                     # [128, 8]

        # Winfo contraction tiles, permuted per core:
        #   tiles 0-3  : my own x-dim blocks (global tile hh*4+j)
        #   tiles 4-11 : gather rows -- tile 4+2j+r is row r (core parity r)
        #                of block j = global x-tile r*4+j; zero when r == hh.
        wiT = g["Winfo"][hs].T.astype(np.float64)   # [1024 xdims, 512 outs]
        wi_tiles = []
        for j in range(4):
            gt = hh * 4 + j
            wi_tiles.append(wiT[gt * 128:(gt + 1) * 128, :])
        for j in range(4):
            for r in range(2):
                if r == hh:
                    wi_tiles.append(np.zeros((128, 512)))
                else:
                    gt = r * 4 + j
                    wi_tiles.append(wiT[gt * 128:(gt + 1) * 128, :])
        wi = np.ascontiguousarray(
            np.concatenate([t[None] for t in wi_tiles], axis=0)
            .transpose(1, 0, 2).reshape(128, 12 * 512).astype(NPBF16))

        ball = np.concatenate([
            bqk("bqa", "bqg"),
            bqk("bka", "bkg"),
            g["bgate"][hs].reshape(4, 128).T.astype(f32),
            binfo_eff[hs].reshape(4, 128).T.astype(f32),
        ], axis=1)

        m = {
            "xqa": xt("query_a"), "xqg": xt("query_g"),
            "xka": xt("key_a"), "xkg": xt("key_g"), "xv": xt("value_a"),
            "wqa": wt("Wqa"), "wqg": wt("Wqg"),
            "wka": wt("Wka"), "wkg": wt("Wkg"), "wv": wt("Wva"),
            "wg": wt("Wgate"), "wi": wi,
            "ball": np.ascontiguousarray(ball.astype(f32)),
        }
        in_maps.append(m)
    return in_maps


def assemble(results):
    out = np.empty((B, S, D), dtype=np.float32)
    for c in range(8):
        b, hh = c // 2, c % 2
        blk = results[c]["out"].reshape(512, S)   # [cols, seq]
        out[b, :, hh * 512:(hh + 1) * 512] = blk.T
    return out


_NC_CACHE = {}


def _get_nc():
    if "nc" not in _NC_CACHE:
        _NC_CACHE["nc"] = build_nc()
    return _NC_CACHE["nc"]


LAST_RESULTS = None


def kernel(**inputs) -> np.ndarray:
    global LAST_RESULTS
    nc = _get_nc()
    in_maps = make_in_maps(inputs)
    trace = os.environ.get("KERNEL_TRACE", "0") == "1"
    kwargs = {}
    if trace:
        kwargs["trace_cores"] = list(range(8))
    res = bass_utils.run_bass_kernel_spmd(
        nc, in_maps, core_ids=list(range(8)), trace=trace, **kwargs,
    )
    LAST_RESULTS = res
    return assemble(res.results)


# revision 22
# speedup vs baseline: 1.0479x; 1.0479x over previous
"""Trainium2 Bass kernel for BoxMultiHeadedAttention (B=4, S=1024, D=1024, H=16).

Reference math (eval mode, mask is all-ones so the masking is a no-op):
    qg/kg/qa/ka/va = per-head projections of the five inputs
    q = concat([qa, qg], -1); k = concat([ka, kg], -1)           # [B,H,S,128]
    p = softmax(q @ k.T / sqrt(128)); x = (p @ va) -> [B,S,D]
    out = sigmoid(concat([query_a, query_g], -1) @ Wgate.T + bgate) * (x @ Winfo.T + binfo)

Sharding: 8 cores = 4 batches x 2 head-halves. Core c handles batch c//2 and
heads (c%2)*8 .. +8 (which are also x-columns (c%2)*512..+512).  The GLU is
column-sharded the same way; attention-output halves are exchanged between
core pairs with per-2-head-block pairwise AllGathers.

v3 schedule:
  * scores in bf16; exp batched over [128,1024] PSUM reads.
  * software pipeline: q/k projections for head h+1 are interleaved into
    attention head h, so the ACT exp chain starts as soon as the first two
    heads are projected instead of after all eight.
  * the recip/normalize chain of head h is deferred into head h+1 (after its
    first exp) so ACT stays dense across head boundaries.
  * prioritized DMA: q/k inputs first in coarse chunks, va/wi/gate weights
    later; all biases packed into one small tensor.
  * info contraction split local/remote: my own x blocks are consumed from
    SBUF (no collective round trip) against host-permuted Winfo tiles; both
    gather rows are contracted (the self-echo row gets zero weights).
"""

import os

import ml_dtypes
import numpy as np

import concourse.bass as bass
import concourse.mybir as mybir
import concourse.tile as tile
from concourse import bacc, bass_utils

B, S, D, H = 4, 1024, 1024, 16
DK = D // H            # 64
CD = 2 * DK            # 128 concat head dim
HL = H // 2            # 8 local heads per core
T = D // 128           # 8 partition tiles per 1024 dim
NQ = S // 512          # 2 moving-dim blocks
SCALE = 1.0 / float(np.sqrt(2 * DK))

F32 = mybir.dt.float32
BF16 = mybir.dt.bfloat16
NPBF16 = ml_dtypes.bfloat16

REPLICA_GROUPS = [[0, 1], [2, 3], [4, 5], [6, 7]]


def build_nc():
    nc = bacc.Bacc("TRN2", target_bir_lowering=False, debug=False, num_devices=8)

    # ---- DRAM I/O (per-core tensors; same program on all 8 cores) ----
    d_xqa = nc.dram_tensor("xqa", [128, T * S], BF16, kind="ExternalInput")
    d_xqg = nc.dram_tensor("xqg", [128, T * S], BF16, kind="ExternalInput")
    d_xka = nc.dram_tensor("xka", [128, T * S], BF16, kind="ExternalInput")
    d_xkg = nc.dram_tensor("xkg", [128, T * S], BF16, kind="ExternalInput")
    d_xv = nc.dram_tensor("xv", [128, T * S], BF16, kind="ExternalInput")
    d_wqa = nc.dram_tensor("wqa", [128, T * 512], BF16, kind="ExternalInput")
    d_wqg = nc.dram_tensor("wqg", [128, T * 512], BF16, kind="ExternalInput")
    d_wka = nc.dram_tensor("wka", [128, T * 512], BF16, kind="ExternalInput")
    d_wkg = nc.dram_tensor("wkg", [128, T * 512], BF16, kind="ExternalInput")
    d_wv = nc.dram_tensor("wv", [128, T * 512], BF16, kind="ExternalInput")
    d_wg = nc.dram_tensor("wg", [128, 2 * T * 512], BF16, kind="ExternalInput")
    # 12 contraction tiles: 0-3 my x blocks, 4-11 the two gather rows per block
    d_wi = nc.dram_tensor("wi", [128, 12 * 512], BF16, kind="ExternalInput")
    # packed biases: cols 0:8 bq, 8:16 bk, 16:20 bgate, 20:24 binfo_eff
    d_ball = nc.dram_tensor("ball", [128, 24], F32, kind="ExternalInput")
    d_out = nc.dram_tensor("out", [4, 128, S], F32, kind="ExternalOutput")

    with tile.TileContext(nc) as tc:
        with (
            tc.tile_pool(name="xin", bufs=1) as p_xin,
            tc.tile_pool(name="wts", bufs=1) as p_w,
            tc.tile_pool(name="big", bufs=1) as p_big,
            tc.tile_pool(name="att", bufs=1) as p_att,
            tc.tile_pool(name="tail", bufs=1) as p_tail,
            tc.tile_pool(name="psA", bufs=1, space="PSUM") as p_psA,
            tc.tile_pool(name="psS", bufs=1, space="PSUM") as p_psS,
            tc.tile_pool(name="psX", bufs=1, space="PSUM") as p_psX,
            tc.tile_pool(name="dram", bufs=1, space="DRAM") as p_dram,
        ):
            # --- persistent sbuf tiles ---
            t_xqa = p_xin.tile([128, T, S], BF16, tag="qin", bufs=2)
            t_xqg = p_xin.tile([128, T, S], BF16, tag="qin", bufs=2)
            t_xka = p_xin.tile([128, T, S], BF16, tag="kin", bufs=2)
            t_xkg = p_xin.tile([128, T, S], BF16, tag="kin", bufs=2)
            t_xv = p_xin.tile([128, T, S], BF16, tag="vin", bufs=1)

            t_wqa = p_w.tile([128, T, 512], BF16, tag="w8", bufs=4)
            t_wqg = p_w.tile([128, T, 512], BF16, tag="w8", bufs=4)
            t_wka = p_w.tile([128, T, 512], BF16, tag="w8", bufs=4)
            t_wkg = p_w.tile([128, T, 512], BF16, tag="w8", bufs=4)
            t_wv = p_w.tile([128, T, 512], BF16, tag="w8", bufs=4)
            t_wi = p_w.tile([128, 12, 512], BF16, tag="wi", bufs=1)
            t_ball = p_w.tile([128, 24], F32, tag="bias", bufs=1)

            t_qT = p_big.tile([128, HL, S], BF16, tag="qk", bufs=2)
            t_kT = p_big.tile([128, HL, S], BF16, tag="qk", bufs=2)

            t_va = p_att.tile([128, T, HL, DK + 1], BF16, tag="va", bufs=1)
            # my normalized x blocks [xdim, block, seq]; lives in a weight
            # slot that frees up once the upfront q-projections finish
            t_xt = p_w.tile([128, 4, S], BF16, tag="w8", bufs=4)
            # gathered remote rows: [xdim, block, row(2), seq]
            t_xr = p_xin.tile([128, 4, 2, S], BF16, tag="kin", bufs=2)
            t_gate = p_big.tile([128, 4, S], BF16, tag="gate", bufs=1)
            # info partial sums (local half, bias folded in)
            t_part = p_big.tile([128, 4, NQ, 512], BF16, tag="part", bufs=1)

            # --- prioritized loads: q/k inputs first, the rest later ---
            def load(dt_, tl, n_t, chunk):
                r = dt_.ap().rearrange("p (t n) -> p t n", t=n_t)
                for tt in range(0, n_t, chunk):
                    nc.sync.dma_start(tl[:, tt:tt + chunk, :],
                                      r[:, tt:tt + chunk, :])

            nc.sync.dma_start(t_ball[:], d_ball.ap())
            load(d_wqa, t_wqa, T, 4)
            load(d_wqg, t_wqg, T, 4)
            load(d_xqa, t_xqa, T, 4)
            load(d_xqg, t_xqg, T, 4)
            load(d_wka, t_wka, T, 4)
            load(d_wkg, t_wkg, T, 4)
            load(d_xka, t_xka, T, 4)
            load(d_xkg, t_xkg, T, 4)
            load(d_wv, t_wv, T, 8)
            load(d_xv, t_xv, T, 4)
            load(d_wi, t_wi, 12, 12)

            # --- PE warmup: keep HAM un-throttled during the DMA lead-in ---
            t_wu = p_att.tile([128, 256], BF16, tag="wu", bufs=1)
            nc.vector.memset(t_wu[:], 0.0)
            for _ in range(16):
                pwu = p_psA.tile([128, 256], F32, tag="proj", bufs=2)
                nc.tensor.matmul(pwu[:], t_wu[:, 0:128], t_wu[:],
                                 start=True, stop=True)

            nc.vector.memset(t_va[:, :, :, DK:DK + 1], 1.0)

            # --- q/k projection groups (concurrent M=64 column-group pairs).
            # gidx: 0=(q,n0) 1=(q,n1) 2=(k,n0) 3=(k,n1) ---
            def proj_group(h, gidx):
                if gidx < 2:
                    wa, wb, xa, xb, dst, boff = (
                        t_wqa, t_wqg, t_xqa, t_xqg, t_qT, 0)
                else:
                    wa, wb, xa, xb, dst, boff = (
                        t_wka, t_wkg, t_xka, t_xkg, t_kT, 8)
                n = gidx % 2
                ps = p_psA.tile([128, 512], F32, tag="proj", bufs=2)
                for kt in range(T):
                    nc.tensor.matmul(
                        ps[0:64, :],
                        wa[:, kt, h * DK:(h + 1) * DK],
                        xa[:, kt, n * 512:(n + 1) * 512],
                        start=(kt == 0), stop=(kt == T - 1),
                        tile_position=(0, 0), skip_group_check=True,
                    )
                    nc.tensor.matmul(
                        ps[64:128, :],
                        wb[:, kt, h * DK:(h + 1) * DK],
                        xb[:, kt, n * 512:(n + 1) * 512],
                        start=(kt == 0), stop=(kt == T - 1),
                        tile_position=(0, 64), skip_group_check=True,
                    )
                nc.vector.tensor_scalar_add(
                    dst[:, h, n * 512:(n + 1) * 512], ps[:],
                    t_ball[:, boff + h:boff + h + 1],
                )

            def va_unit(st):
                ps = p_psA.tile([128, 512], F32, tag="proj", bufs=2)
                for kt in range(T):
                    nc.tensor.matmul(
                        ps[:],
                        t_xv[:, kt, st * 128:(st + 1) * 128],
                        t_wv[:, kt, :],
                        start=(kt == 0), stop=(kt == T - 1),
                    )
                nc.vector.tensor_copy(
                    t_va[:, st, :, 0:DK],
                    ps[:].rearrange("p (h d) -> p h d", h=HL),
                )

            # gate weights reuse the xv slot once va is done; loaded lazily
            t_wg = p_xin.tile([128, 2 * T, 512], BF16, tag="vin", bufs=1)
            _wg_loaded = [False]

            def gate_unit(mt, n):
                if not _wg_loaded[0]:
                    _wg_loaded[0] = True
                    load(d_wg, t_wg, 2 * T, 16)
                ps = p_psA.tile([128, 512], F32, tag="proj", bufs=2)
                for kt in range(2 * T):
                    xsrc = t_xqa if kt < T else t_xqg
                    nc.tensor.matmul(
                        ps[:],
                        t_wg[:, kt, mt * 128:(mt + 1) * 128],
                        xsrc[:, kt % T, n * 512:(n + 1) * 512],
                        start=(kt == 0), stop=(kt == 2 * T - 1),
                    )
                nc.vector.tensor_scalar_add(
                    t_gate[:, mt, n * 512:(n + 1) * 512], ps[:],
                    t_ball[:, 16 + mt:17 + mt],
                )

            gate_q = [(mt, n) for mt in range(4) for n in range(NQ)]
            _gate_pos = [0]

            def emit_gate(k=1):
                while k > 0 and _gate_pos[0] < len(gate_q):
                    mt, n = gate_q[_gate_pos[0]]
                    _gate_pos[0] += 1
                    gate_unit(mt, n)
                    k -= 1

            # local-info partials: one matmul + one DVE accumulate per
            # (block, mt, n).  Bias folded in at block 0.
            def local_info_block(b):
                for mt in range(4):
                    for n in range(NQ):
                        ps = p_psA.tile([128, 512], F32, tag="proj", bufs=2)
                        nc.tensor.matmul(
                            ps[:],
                            t_wi[:, b, mt * 128:(mt + 1) * 128],
                            t_xt[:, b, n * 512:(n + 1) * 512],
                            start=True, stop=True,
                        )
                        if b == 0:
                            nc.vector.tensor_scalar_add(
                                t_part[:, mt, n, :], ps[:],
                                t_ball[:, 20 + mt:21 + mt],
                            )
                        else:
                            nc.vector.tensor_tensor(
                                t_part[:, mt, n, :], ps[:],
                                t_part[:, mt, n, :], op=mybir.AluOpType.add,
                            )

            def ship_block(i):
                cc_in = p_dram.tile([1, 128, S], BF16, name=f"cci_{i}")
                cc_out = p_dram.tile([2, 128, S], BF16, name=f"cco_{i}")
                nc.sync.dma_start(cc_in[0], t_xt[:, i, :])
                nc.gpsimd.collective_compute(
                    "AllGather", mybir.AluOpType.bypass,
                    replica_groups=REPLICA_GROUPS,
                    ins=[cc_in[:].opt()], outs=[cc_out[:].opt()],
                )
                nc.sync.dma_start(t_xr[:, i, 0, :], cc_out[0])
                nc.sync.dma_start(t_xr[:, i, 1, :], cc_out[1])

            # per-block finisher: both heads' recip chains batched so the
            # ln/exp table switches cost 2 loads per block, not per head.
            def make_block_finisher(j, tpx_e, tpx_o):
                def fin():
                    t_ln = p_att.tile([33, S], F32, tag="recip", bufs=1,
                                      name=f"ln_{j}")
                    nc.scalar.activation(t_ln[0:1, :], tpx_e[DK:DK + 1, :],
                                         mybir.ActivationFunctionType.Ln)
                    nc.scalar.activation(t_ln[32:33, :], tpx_o[DK:DK + 1, :],
                                         mybir.ActivationFunctionType.Ln)
                    recs = []
                    for par in (0, 1):
                        t_recip = p_att.tile([1, S], BF16, tag="recip2",
                                             bufs=2, name=f"recip_{j}_{par}")
                        nc.scalar.activation(t_recip[:],
                                             t_ln[32 * par:32 * par + 1, :],
                                             mybir.ActivationFunctionType.Exp,
                                             scale=-1.0)
                        recs.append(t_recip)
                    t_bc = p_att.tile([DK, S], BF16, tag="bc", bufs=1,
                                      name=f"bc_{j}")
                    for par, tpx in ((0, tpx_e), (1, tpx_o)):
                        nc.gpsimd.partition_broadcast(t_bc[:], recs[par][:])
                        nc.vector.tensor_tensor(
                            t_xt[par * DK:par * DK + DK, j, :],
                            tpx[0:DK, :], t_bc[:], op=mybir.AluOpType.mult,
                        )
                    ship_block(j)
                    local_info_block(j)
                return fin

            pend = []

            def attn_head(h):
                px = p_psX.tile([DK + 1, S], F32, tag="x", bufs=1,
                                name=f"px_{h}")
                tes = []
                pv_done = [0]

                def pv(j):
                    for n in range(NQ):
                        nc.tensor.matmul(
                            px[:, n * 512:(n + 1) * 512],
                            t_va[:, j, h, :],
                            tes[j][:, n * 512:(n + 1) * 512],
                            start=(j == 0), stop=(j == T - 1),
                        )
                    pv_done[0] = j + 1

                for kt in range(T):
                    pss = p_psS.tile([128, S], F32, tag="s", bufs=2,
                                     name=f"pss_{h}_{kt}")
                    for n in range(NQ):
                        nc.tensor.matmul(
                            pss[:, n * 512:(n + 1) * 512],
                            t_kT[:, h, kt * 128:(kt + 1) * 128],
                            t_qT[:, h, n * 512:(n + 1) * 512],
                            start=True, stop=True,
                        )
                    te = p_att.tile([128, S], BF16, tag="exp", bufs=3,
                                    name=f"te_{h}_{kt}")
                    nc.scalar.activation(
                        te[:], pss[:],
                        mybir.ActivationFunctionType.Exp, scale=SCALE,
                    )
                    tes.append(te)
                    if kt == 1 and pend:
                        pend.pop(0)()
                    if h == 0:
                        # va tiles are produced just in time inside head 0;
                        # p@v for key-tile j must follow va tile j in the PE
                        # queue.
                        if kt == 2:
                            va_unit(0); va_unit(1); va_unit(2)
                            pv(0); pv(1)
                        elif kt >= 3:
                            va_unit(kt)
                            pv(kt - 1)
                    else:
                        if kt > 0:
                            pv(kt - 1)
                        # pipeline: k-projections of heads 2-7 run inside
                        # heads 1-3; gate preacts inside heads 4-7
                        if 1 <= h <= 3:
                            nh = 2 * h
                            if kt == 1:
                                proj_group(nh, 2)
                            elif kt == 3:
                                proj_group(nh, 3)
                            elif kt == 5:
                                proj_group(nh + 1, 2)
                            elif kt == 6:
                                proj_group(nh + 1, 3)
                        if h >= 4 and kt in (2, 5):
                            emit_gate(1)
                while pv_done[0] < T:
                    pv(pv_done[0])
                # copy px out of PSUM immediately so its bank pair recycles
                t_px = p_att.tile([DK + 1, S], BF16, tag="pxs", bufs=3,
                                  name=f"pxs_{h}")
                nc.vector.tensor_copy(t_px[:], px[:])
                return t_px

            # --- pipeline: all q-projections + k h0,h1 upfront (overlapping
            # the DMA lead-in), then attention with inline k-projections ---
            for h in range(HL):
                proj_group(h, 0)
                proj_group(h, 1)
            for h in (0, 1):
                proj_group(h, 2)
                proj_group(h, 3)
            tpx_prev = None
            for h in range(HL):
                tpx = attn_head(h)
                if h % 2 == 1:
                    pend.append(make_block_finisher(h // 2, tpx_prev, tpx))
                tpx_prev = tpx
            while pend:
                pend.pop(0)()   # finishers for blocks 2,3

            # --- tail: batched gate sigmoids, remote info, GLU, store ---
            emit_gate(8)
            for mt in range(4):
                nc.scalar.activation(
                    t_gate[:, mt, :], t_gate[:, mt, :],
                    mybir.ActivationFunctionType.Sigmoid,
                )

            for mt in range(4):
                for n in range(NQ):
                    ps = p_psA.tile([128, 512], F32, tag="proj", bufs=2)
                    k = 0
                    for j in range(4):
                        for r in range(2):
                            nc.tensor.matmul(
                                ps[:],
                                t_wi[:, 4 + 2 * j + r, mt * 128:(mt + 1) * 128],
                                t_xr[:, j, r, n * 512:(n + 1) * 512],
                                start=(k == 0), stop=(k == 7),
                            )
                            k += 1
                    t_ob = p_tail.tile([128, 512], F32, tag="outb", bufs=2)
                    nc.vector.tensor_tensor(
                        t_ob[:], ps[:], t_part[:, mt, n, :],
                        op=mybir.AluOpType.add,
                    )
                    nc.vector.tensor_tensor(
                        t_ob[:], t_ob[:],
                        t_gate[:, mt, n * 512:(n + 1) * 512],
                        op=mybir.AluOpType.mult,
                    )
                    nc.sync.dma_start(
                        d_out.ap()[mt, :, n * 512:(n + 1) * 512], t_ob[:])

    nc.compile()
    return nc


def make_in_maps(inputs):
    """Host-side sharding: transpose/slice/cast the full inputs per core."""
    f32 = np.float32
    g = {k: np.asarray(v) for k, v in inputs.items()}
    binfo_eff = (
        g["binfo"].astype(np.float64)
        + g["Winfo"].astype(np.float64) @ g["bva"].astype(np.float64)
    ).astype(f32)

    in_maps = []
    for c in range(8):
        b, hh = c // 2, c % 2
        hs = slice(hh * 512, (hh + 1) * 512)

        def pmajor(a):
            rows, n = a.shape
            t = rows // 128
            return np.ascontiguousarray(
                a.reshape(t, 128, n).transpose(1, 0, 2).reshape(128, t * n))

        def xt(name):
            return pmajor(g[name][b].T.astype(NPBF16))

        def wt(name):
            return pmajor(g[name][hs].T.astype(NPBF16))

        def bqk(pa, pg):
            a = g[pa][hs].reshape(HL, DK).T.astype(f32)   # [64, 8]
            gg = g[pg][hs].reshape(HL, DK).T.astype(f32)
            return np.vstack([a, gg])                     # [128, 8]

        # Winfo contraction tiles, permuted per core:
        #   tiles 0-3  : my own x-dim blocks (global tile hh*4+j)
        #   tiles 4-11 : gather rows -- tile 4+2j+r is row r (core parity r)
        #                of block j = global x-tile r*4+j; zero when r == hh.
        wiT = g["Winfo"][hs].T.astype(np.float64)   # [1024 xdims, 512 outs]
        wi_tiles = []
        for j in range(4):
            gt = hh * 4 + j
            wi_tiles.append(wiT[gt * 128:(gt + 1) * 128, :])
        for j in range(4):
            for r in range(2):
                if r == hh:
                    wi_tiles.append(np.zeros((128, 512)))
                else:
                    gt = r * 4 + j
                    wi_tiles.append(wiT[gt * 128:(gt + 1) * 128, :])
        wi = np.ascontiguousarray(
            np.concatenate([t[None] for t in wi_tiles], axis=0)
            .transpose(1, 0, 2).reshape(128, 12 * 512).astype(NPBF16))

        ball = np.concatenate([
            bqk("bqa", "bqg"),
            bqk("bka", "bkg"),
            g["bgate"][hs].reshape(4, 128).T.astype(f32),
            binfo_eff[hs].reshape(4, 128).T.astype(f32),
        ], axis=1)

        m = {
            "xqa": xt("query_a"), "xqg": xt("query_g"),
            "xka": xt("key_a"), "xkg": xt("key_g"), "xv": xt("value_a"),
            "wqa": wt("Wqa"), "wqg": wt("Wqg"),
            "wka": wt("Wka"), "wkg": wt("Wkg"), "wv": wt("Wva"),
            "wg": wt("Wgate"), "wi": wi,
            "ball": np.ascontiguousarray(ball.astype(f32)),
        }
        in_maps.append(m)
    return in_maps


def assemble(results):
    out = np.empty((B, S, D), dtype=np.float32)
    for c in range(8):
        b, hh = c // 2, c % 2
        blk = results[c]["out"].reshape(512, S)   # [cols, seq]
        out[b, :, hh * 512:(hh + 1) * 512] = blk.T
    return out


_NC_CACHE = {}


def _get_nc():
    if "nc" not in _NC_CACHE:
        _NC_CACHE["nc"] = build_nc()
    return _NC_CACHE["nc"]


LAST_RESULTS = None


def kernel(**inputs) -> np.ndarray:
    global LAST_RESULTS
    nc = _get_nc()
    in_maps = make_in_maps(inputs)
    trace = os.environ.get("KERNEL_TRACE", "0") == "1"
    kwargs = {}
    if trace:
        kwargs["trace_cores"] = list(range(8))
    res = bass_utils.run_bass_kernel_spmd(
        nc, in_maps, core_ids=list(range(8)), trace=trace, **kwargs,
    )
    LAST_RESULTS = res
    return assemble(res.results)
